# revision 1
# baseline (speedup 1.0000x reference)
"""Trainium2 Bass kernel for nn_ContrastiveLoss (wav2vec2-style contrastive loss).

Shapes (hardcoded): B=8, C=256, T=1024, M=512 masked positions, K=100 negatives.
Sharding: pure data parallel — batch row b -> NeuronCore b (8 cores).

Per core the dominant work is streaming negatives[b] ([M, K, C] f32, 52.4 MB)
from HBM once, computing per (m, k):
    dot[m,k]   = sum_c neg[m,k,c] * ctx_m[m,c]
    sumsq[m,k] = sum_c neg[m,k,c]^2
then cosine normalization, logsumexp over K+1 logits, per-row loss.

Engine split: VectorE does all dots (fused scalar_tensor_tensor multiply +
accumulate) plus a small share of the sumsq; ScalarE does the rest of the
sumsq (activation Square with accum_out, outputs in PSUM which has lower
per-op overhead for ScalarE). Epilogues for all 4 m-groups run at the end,
grouped by activation function to avoid ACT table reloads.
The device returns per-row losses [128, 4] per core; the host sums & divides.
"""

import numpy as np

TEMP = 0.1
EPS = 1e-8
B, C, T = 8, 256, 1024
M = 512  # masked positions per batch row
K = 100  # negatives per masked position
P = 128  # partitions
G = M // P  # m-groups per core (4)
KCH = 10  # k's per streamed tile: [128, KCH, C] f32 = 1.25 MB
NKC = K // KCH  # stream tiles per m-group (10)
SPLIT = 5  # k % SPLIT == 0 -> sumsq on VectorE, else ScalarE

_NC = None


def _build_nc():
    import concourse.bacc as bacc
    import concourse.tile as tile
    from concourse import mybir

    f32 = mybir.dt.float32
    Alu = mybir.AluOpType
    Act = mybir.ActivationFunctionType

    nc = bacc.Bacc(trn_type="TRN2")
    neg = nc.dram_tensor("neg", [M, K, C], f32, kind="ExternalInput")
    ctxg = nc.dram_tensor("ctxg", [M, C], f32, kind="ExternalInput")
    posg = nc.dram_tensor("posg", [M, C], f32, kind="ExternalInput")
    rowloss = nc.dram_tensor("rowloss", [P, G], f32, kind="ExternalOutput")

    with tile.TileContext(nc) as tc:
        with (
            tc.tile_pool(name="stream", bufs=5) as stream,
            tc.tile_pool(name="grp", bufs=2) as grp,
            tc.tile_pool(name="pg", bufs=G) as pg,
            tc.tile_pool(name="scrp", bufs=2) as scrp,
            tc.tile_pool(name="psg", bufs=G) as psg,
            tc.tile_pool(name="pss", bufs=2) as pss,
            tc.tile_pool(name="outp", bufs=1) as outp,
        ):
            out_t = outp.tile([P, G], f32)
            # per-group persistent tiles (epilogue runs after all streaming)
            gt = {}
            for g in range(G):
                gt[g] = dict(
                    css=pg.tile([P, 1], f32, tag="css", name=f"css{g}"),
                    pss_t=pg.tile([P, 1], f32, tag="pss_t", name=f"pss_t{g}"),
                    cpd=pg.tile([P, 1], f32, tag="cpd", name=f"cpd{g}"),
                    rawdots=pg.tile([P, K], f32, tag="rawdots", name=f"rawdots{g}"),
                    negss=psg.tile([P, K], f32, tag="negss", name=f"negss{g}"),
                    logits=pg.tile([P, K + 1], f32, tag="logits", name=f"logits{g}"),
                )

            for g in range(G):
                m0 = g * P
                d = gt[g]
                ctx_t = grp.tile([P, C], f32, tag="ctx")
                pos_t = grp.tile([P, C], f32, tag="pos")
                nc.sync.dma_start(out=ctx_t[:], in_=ctxg[m0 : m0 + P, :])
                nc.sync.dma_start(out=pos_t[:], in_=posg[m0 : m0 + P, :])

                scr = scrp.tile([P, C], f32, tag="scr")
                nc.vector.scalar_tensor_tensor(
                    out=scr[:], in0=ctx_t[:], scalar=1.0, in1=ctx_t[:],
                    op0=Alu.mult, op1=Alu.mult, accum_out=d["css"][:],
                )
                nc.vector.scalar_tensor_tensor(
                    out=scr[:], in0=pos_t[:], scalar=1.0, in1=pos_t[:],
                    op0=Alu.mult, op1=Alu.mult, accum_out=d["pss_t"][:],
                )
                nc.vector.scalar_tensor_tensor(
                    out=scr[:], in0=ctx_t[:], scalar=1.0, in1=pos_t[:],
                    op0=Alu.mult, op1=Alu.mult, accum_out=d["cpd"][:],
                )

                for t in range(NKC):
                    nt = stream.tile([P, KCH, C], f32, tag="nt")
                    nc.sync.dma_start(
                        out=nt[:],
                        in_=neg[m0 : m0 + P, t * KCH : (t + 1) * KCH, :],
                    )
                    for j in range(KCH):
                        k = t * KCH + j
                        nc.vector.scalar_tensor_tensor(
                            out=scr[:], in0=nt[:, j, :], scalar=1.0, in1=ctx_t[:],
                            op0=Alu.mult, op1=Alu.mult,
                            accum_out=d["rawdots"][:, k : k + 1],
                        )
                        if k % SPLIT == 0:
                            nc.vector.scalar_tensor_tensor(
                                out=scr[:], in0=nt[:, j, :], scalar=1.0,
                                in1=nt[:, j, :], op0=Alu.mult, op1=Alu.mult,
                                accum_out=d["negss"][:, k : k + 1],
                            )
                        else:
                            scr2 = pss.tile([P, C], f32, tag="scr2")
                            nc.scalar.activation(
                                out=scr2[:], in_=nt[:, j, :], func=Act.Square,
                                accum_out=d["negss"][:, k : k + 1],
                            )

            # ---- batched epilogue, grouped by ACT function ----
            crn, prn, nrn, mx, mxs, se, lnse, t1 = {}, {}, {}, {}, {}, {}, {}, {}
            for g in range(G):
                d = gt[g]
                crn[g] = pg.tile([P, 1], f32, tag="crn", name=f"crn{g}")
                prn[g] = pg.tile([P, 1], f32, tag="prn", name=f"prn{g}")
                nrn[g] = pg.tile([P, K], f32, tag="nrn", name=f"nrn{g}")
            # all sqrts first (one Sqrt table load)
            for g in range(G):
                d = gt[g]
                nc.scalar.sqrt(d["css"][:], d["css"][:])
                nc.scalar.sqrt(d["pss_t"][:], d["pss_t"][:])
                nc.scalar.sqrt(d["negss"][:], d["negss"][:])
            for g in range(G):
                d = gt[g]
                nc.vector.tensor_scalar_max(d["css"][:], d["css"][:], EPS)
                nc.vector.tensor_scalar_max(d["pss_t"][:], d["pss_t"][:], EPS)
                nc.vector.tensor_scalar_max(d["negss"][:], d["negss"][:], EPS)
                nc.vector.reciprocal(crn[g][:], d["css"][:])
                nc.vector.reciprocal(prn[g][:], d["pss_t"][:])
                nc.vector.reciprocal(nrn[g][:], d["negss"][:])
                # logits: col 0 = positive sim, cols 1..K = negative sims
                nc.vector.scalar_tensor_tensor(
                    out=d["logits"][:, 0:1], in0=d["cpd"][:], scalar=crn[g][:],
                    in1=prn[g][:], op0=Alu.mult, op1=Alu.mult,
                )
                nc.vector.scalar_tensor_tensor(
                    out=d["logits"][:, 1 : K + 1], in0=d["rawdots"][:],
                    scalar=crn[g][:], in1=nrn[g][:], op0=Alu.mult, op1=Alu.mult,
                )
                mx[g] = pg.tile([P, 1], f32, tag="mx", name=f"mx{g}")
                mxs[g] = pg.tile([P, 1], f32, tag="mxs", name=f"mxs{g}")
                nc.vector.reduce_max(
                    mx[g][:], d["logits"][:], axis=mybir.AxisListType.X
                )
                nc.vector.tensor_scalar_mul(mxs[g][:], mx[g][:], -1.0 / TEMP)
            # all exps (one Exp table load)
            for g in range(G):
                d = gt[g]
                esc = scrp.tile([P, K + 1], f32, tag="esc")
                se[g] = pg.tile([P, 1], f32, tag="se", name=f"se{g}")
                nc.scalar.activation(
                    out=esc[:], in_=d["logits"][:], func=Act.Exp,
                    scale=1.0 / TEMP, bias=mxs[g][:], accum_out=se[g][:],
                )
            # all lns (one Ln table load)
            for g in range(G):
                lnse[g] = pg.tile([P, 1], f32, tag="lnse", name=f"lnse{g}")
                nc.scalar.activation(out=lnse[g][:], in_=se[g][:], func=Act.Ln)
            for g in range(G):
                d = gt[g]
                t1[g] = pg.tile([P, 1], f32, tag="t1", name=f"t1{g}")
                nc.vector.scalar_tensor_tensor(
                    out=t1[g][:], in0=mx[g][:], scalar=1.0 / TEMP, in1=lnse[g][:],
                    op0=Alu.mult, op1=Alu.add,
                )
                nc.vector.scalar_tensor_tensor(
                    out=out_t[:, g : g + 1], in0=d["logits"][:, 0:1],
                    scalar=-1.0 / TEMP, in1=t1[g][:], op0=Alu.mult, op1=Alu.add,
                )
            nc.sync.dma_start(out=rowloss[:], in_=out_t[:])
    nc.finalize()
    return nc


def _get_nc():
    global _NC
    if _NC is None:
        _NC = _build_nc()
    return _NC


def kernel(context, positive, negatives, mask_indices, num_masked):
    from concourse.bass_utils import run_bass_kernel_spmd

    context = np.asarray(context, dtype=np.float32)
    positive = np.asarray(positive, dtype=np.float32)
    negatives = np.asarray(negatives, dtype=np.float32)
    mask = np.asarray(mask_indices).astype(bool)
    nm = int(np.asarray(num_masked))
    assert nm == M, f"kernel hardcodes num_masked={M}, got {nm}"
    assert context.shape == (B, C, T) and negatives.shape == (B, M, K, C)

    in_maps = []
    for b in range(B):
        idx = np.flatnonzero(mask[b])
        assert idx.size == M, f"row {b}: expected {M} masked, got {idx.size}"
        ctxg = np.ascontiguousarray(context[b].T[idx])  # [M, C]
        posg = np.ascontiguousarray(positive[b].T[idx])  # [M, C]
        in_maps.append(
            {
                "neg": np.ascontiguousarray(negatives[b]),
                "ctxg": ctxg,
                "posg": posg,
            }
        )

    res = run_bass_kernel_spmd(_get_nc(), in_maps, core_ids=list(range(B)))
    total = np.float64(0.0)
    for r in res.results:
        total += r["rowloss"].astype(np.float64).sum()
    return np.float32(total / (B * M))

